# revision 3
# baseline (speedup 1.0000x reference)
"""Batched Viterbi decode (CRF) on 8 Trainium2 NeuronCores — v3.

Same contract and math as kernel.py (bitwise-exact fp32 DP value history,
host backtrack). Differences vs baseline:
  - per-group add engines configurable (ACT / DVE / GPSIMD)
  - reduce may be split [first-k | tail] to shorten the critical chain
  - em adds may run on GPSIMD to relieve DVE
  - emission order tuned so each group's slow ACT adds overlap PE/DVE work

Per step, per 4-seq group g:
  scores_s[j,i] = trans_s[j,i] + t1_{t-1,s}[j]     (engine per ADD_ENG)
  psum_s = scores_s^T                               (PE transpose)
  m[:,s] = max_j psum[:, s, j]                      (DVE reduce, split cfg)
  t1hist[:, t*8+g*4+s] = m[:,s] + em_cols[:, ...]   (EM_ENG)
"""

import os
from contextlib import ExitStack

import numpy as np

S = 128
T = 2048
NS = 8
N_CORES = 8
B = NS * N_CORES

# knobs
ADD_ENG = os.environ.get("V3_ADD", "vaavaaaa")  # per-seq: a=ACT v=DVE p=POOL
EM_ENG = os.environ.get("V3_EM", "p")           # v=DVE p=POOL
RED_SPLIT = int(os.environ.get("V3_RSPLIT", "0"))  # 0: [4]; 3: [3|1]; 2: [2|2]
PE_FILLER = int(os.environ.get("V3_FILLER", "0"))  # dummy transposes/group to keep PE clocked up
# groups: comma-separated seq-count per reduce group, e.g. "3,3,2"
GROUPS = [int(x) for x in os.environ.get("V3_GROUPS", "3,3,2").split(",")]
assert sum(GROUPS) == NS

_CACHE = {}


def _build_forward():
    import concourse.bacc as bacc
    import concourse.mybir as mybir
    import concourse.tile as tile

    F32 = mybir.dt.float32
    ALU = mybir.AluOpType
    AX = mybir.AxisListType

    nc = bacc.Bacc("TRN2", num_devices=N_CORES)
    trans_in = nc.dram_tensor("transitions", [NS, S + 1, S], F32, kind="ExternalInput")
    em_in = nc.dram_tensor("emissions", [NS, T, S], F32, kind="ExternalInput")
    ident_in = nc.dram_tensor("identity", [S, S], F32, kind="ExternalInput")
    t1_out = nc.dram_tensor("t1hist", [S, T * NS], F32, kind="ExternalOutput")

    with ExitStack() as ctx:
        trans_sb = ctx.enter_context(nc.sbuf_tensor([S, NS * S], F32))
        em_cols = ctx.enter_context(nc.sbuf_tensor([S, T * NS], F32))
        t1hist = ctx.enter_context(nc.sbuf_tensor([S, T * NS], F32))
        ident = ctx.enter_context(nc.sbuf_tensor([S, S], F32))
        start_sb = ctx.enter_context(nc.sbuf_tensor([S, NS], F32))
        em0_sb = ctx.enter_context(nc.sbuf_tensor([S, NS], F32))
        gbase = [sum(GROUPS[:i]) for i in range(len(GROUPS))]
        psum_pp = [
            [
                ctx.enter_context(nc.psum_tensor(f"pspp{k}g{g}", [S, GROUPS[g] * S], F32))
                for g in range(len(GROUPS))
            ]
            for k in range(2)
        ]

        with tile.TileContext(nc) as tc, ExitStack() as pctx:
            sc_pool = pctx.enter_context(tc.tile_pool(name="scores", bufs=3))
            tmp_pool = pctx.enter_context(tc.tile_pool(name="tmp", bufs=3))
            stage_pool = pctx.enter_context(tc.tile_pool(name="stage", bufs=4))
            pst_pool = pctx.enter_context(tc.tile_pool(name="pst", bufs=2, space="PSUM"))

            # ---- prologue ----
            for s in range(NS):
                nc.sync.dma_start(trans_sb[:, s * S:(s + 1) * S], trans_in[s, 0:S, :])
            nc.sync.dma_start(ident[:], ident_in[:])
            for s in range(NS):
                nc.sync.dma_start(
                    start_sb[:, s:s + 1], trans_in[s, S:S + 1, :].rearrange("o p -> p o")
                )
                nc.sync.dma_start(
                    em0_sb[:, s:s + 1], em_in[s, 0:1, :].rearrange("o p -> p o")
                )
            nc.vector.tensor_add(t1hist[:, 0:NS], start_sb[:], em0_sb[:])

            for s in range(NS):
                for c in range(T // S):
                    stage = stage_pool.tile([S, S], F32, tag="emstage")
                    nc.sync.dma_start(stage[:], em_in[s, c * S:(c + 1) * S, :])
                    pst = pst_pool.tile([S, S], F32, tag="empsum")
                    nc.tensor.transpose(pst[:], stage[:], ident[:])
                    dst = em_cols[:, c * S * NS + s: (c + 1) * S * NS: NS]
                    nc.scalar.copy(dst, pst[:])

            # ---- main DP loop ----
            def emit_add(sc, s, t):
                t1col = t1hist[:, (t - 1) * NS + s:(t - 1) * NS + s + 1]
                src = trans_sb[:, s * S:(s + 1) * S]
                gb = max(b for b in gbase if b <= s)
                dst = sc[:, (s - gb) * S:(s - gb) * S + S]
                e = ADD_ENG[s]
                if e == "a":
                    nc.scalar.activation(
                        dst, src, mybir.ActivationFunctionType.Identity,
                        bias=t1col, scale=1.0,
                    )
                elif e == "v":
                    nc.vector.tensor_scalar_add(dst, src, t1col)
                else:
                    nc.gpsimd.tensor_scalar_add(dst, src, t1col)

            def emit_em(dst, m, em):
                if EM_ENG == "v":
                    nc.vector.tensor_add(dst, m, em)
                else:
                    nc.gpsimd.tensor_add(dst, m, em)

            def step(t):
                k = t % 2
                scs = [sc_pool.tile([S, GROUPS[g] * S], F32, name=f"sc{g}", tag=f"sc{g}")
                       for g in range(len(GROUPS))]

                def seq_slot(s):
                    for g in range(len(GROUPS)):
                        if s < gbase[g] + GROUPS[g]:
                            return g, s - gbase[g]

                # per-group emission: v-add first, then ACT adds (2 serial max)
                for g in range(len(GROUPS)):
                    gs = list(range(gbase[g], gbase[g] + GROUPS[g]))
                    gs.sort(key=lambda s: {"p": 0, "v": 1, "a": 2}[ADD_ENG[s]])
                    for s in gs:
                        _, sl = seq_slot(s)
                        emit_add(scs[g], gbase[g] + sl if False else s, t)
                tmp = tmp_pool.tile([S, NS], F32, name="tmp", tag="tmp")
                for g in range(len(GROUPS)):
                    pst = psum_pp[k][g]
                    gs = list(range(gbase[g], gbase[g] + GROUPS[g]))
                    gs.sort(key=lambda s: {"p": 0, "v": 1, "a": 2}[ADD_ENG[s]])
                    for s in gs:
                        sl = s - gbase[g]
                        nc.tensor.transpose(
                            pst[:, sl * S:(sl + 1) * S],
                            scs[g][:, sl * S:(sl + 1) * S], ident[:])
                    pg = pst[:].rearrange("p (s i) -> p s i", i=S)
                    lo = gbase[g]
                    nc.vector.tensor_reduce(
                        tmp[:, lo:lo + GROUPS[g]], pg, axis=AX.X, op=ALU.max)
                    emit_em(
                        t1hist[:, t * NS + lo:t * NS + lo + GROUPS[g]],
                        tmp[:, lo:lo + GROUPS[g]],
                        em_cols[:, t * NS + lo:t * NS + lo + GROUPS[g]])

            for t in range(1, T):
                step(t)

            # ---- epilogue ----
            n_dma = 8
            cols = T * NS // n_dma
            for d in range(n_dma):
                nc.sync.dma_start(
                    t1_out[:, d * cols:(d + 1) * cols],
                    t1hist[:, d * cols:(d + 1) * cols])

    nc.finalize()
    return nc


def _get_nc():
    if "nc" not in _CACHE:
        _CACHE["nc"] = _build_forward()
    return _CACHE["nc"]


def kernel(transitions, emissions, lengths):
    from concourse.bass_utils import run_bass_kernel_spmd

    transitions = np.ascontiguousarray(transitions, dtype=np.float32)
    emissions = np.ascontiguousarray(emissions, dtype=np.float32)
    lengths = np.asarray(lengths, dtype=np.int32)
    assert transitions.shape == (B, S + 1, S)
    assert emissions.shape == (B, T, S)

    nc = _get_nc()
    eye = np.eye(S, dtype=np.float32)
    in_maps = [
        {
            "transitions": transitions[c * NS:(c + 1) * NS],
            "emissions": emissions[c * NS:(c + 1) * NS],
            "identity": eye,
        }
        for c in range(N_CORES)
    ]
    res = run_bass_kernel_spmd(
        nc, in_maps, core_ids=list(range(N_CORES)),
        trace=bool(os.environ.get("VIT_TRACE")),
    )
    if os.environ.get("VIT_TRACE"):
        _CACHE["last_exec_time_ns"] = res.exec_time_ns
        _CACHE["insts_and_trace"] = res.instructions_and_trace

    t1 = np.empty((B, T, S), dtype=np.float32)
    for c in range(N_CORES):
        t1[c * NS:(c + 1) * NS] = (
            res.results[c]["t1hist"].reshape(S, T, NS).transpose(2, 1, 0)
        )

    return _backtrack(transitions, emissions, lengths, t1)


def _backtrack(transitions, emissions, lengths, t1):
    trans = transitions[:, :S, :]
    nb = np.arange(B)
    z = np.zeros((B, T), dtype=np.int32)
    last = lengths - 1
    z_last = np.argmax(t1[nb, last, :], axis=1).astype(np.int32)
    ptr = z_last.copy()
    for t in range(int(last.max()), 0, -1):
        at_last = (t == last)
        if at_last.any():
            ptr = np.where(at_last, z_last, ptr)
        z[:, t] = np.where(t <= last, ptr, 0)
        col = (t1[:, t - 1, :] + trans[nb, :, ptr]) + emissions[nb, t, ptr][:, None]
        ptr_new = np.argmax(col, axis=1).astype(np.int32)
        ptr = np.where(t <= last, ptr_new, ptr)
    z[:, 0] = ptr
    return z
